# revision 13
# baseline (speedup 1.0000x reference)
"""8-core Trainium2 Bass kernel for nn_Attention_54778012893378.

Tensor-parallel over heads (2 heads/core). v4: fully 16-bit PE datapath
(fp16 for x/weights/q/k/scores, bf16 where exp range demands it), with
attention emitted tile-by-tile INSIDE the projection dt-loops. The exp on
ScalarE (685ns/tile) outweighs the PE work per attention tile (426ns), so
attention alone stalls the in-order PE queue; interleaving projection
matmuls between attention tiles keeps the PE saturated and the HAM clock
gate at the full 2.4 GHz (idle gaps re-throttle the PE to 1.2 GHz).

Schedule per seq chunk sc (512 positions):
  q-chain (x@wq, both heads) with head-1/q-chunk sc-1 attention tiles
  between dt steps; rope_q on DVE; k0-chain then k1-chain (k1 waits
  rope_q0's psum ring slot, which is free by then) with head-0/q-chunk sc
  off-band attention tiles between dt steps; rope_k0; v st-chains with
  head-0 diagonal-band tiles between them (rope_k1 + v copies interleaved
  on DVE so nothing stalls).
Head-0's AllToAll fires inside the last chunk; head-1/q-chunk 3 is the only
post-loop attention, overlapping that collective; the two wo halves overlap
head-1's AllToAll. wo streams on gpsimd early (the collective triggers
queue behind it harmlessly); a second tiny warm-up collective ensures the
first real AllToAll runs on a warmed CC path. Scores are scaled via the
per-query factor folded into q's rope tables (single shared table when
seq_scale is uniform); the causal mask is 4 shifted slices of one bf16
staircase tile, applied as a multiply after exp; the softmax denominator
accumulates in bf16 on DVE and broadcasts via one ones-matmul.
"""

import numpy as np
import ml_dtypes

import concourse.bass as bass
import concourse.bacc as bacc
import concourse.tile as tile
import concourse.mybir as mybir
from concourse.bass_utils import run_bass_kernel_spmd

F32 = mybir.dt.float32
F32R = mybir.dt.float32r
F16 = mybir.dt.float16
BF16 = mybir.dt.bfloat16
AF = mybir.ActivationFunctionType
bf16 = ml_dtypes.bfloat16

# problem dims (hardcoded per spec)
S, D, H, HD, NC = 2048, 2048, 16, 128, 8
HL = H // NC            # local heads per core
CW = HL * HD            # per-core head-column width
RW = S // NC            # per-core output row width


def _rope_drain(nc, rtmp, ps, out_sl, cs, cs_w, w):
    """Full-width rope from psum [te;to]: oe = te*c - to*s ; oo = te*s + to*c.
    cs = [c;s] packed [128, w]; cs_w = [s;c] (swapped). Mixed psum+sbuf
    operands may differ in base partition; both-sbuf operands may not, so
    the upper halves are staged through base-0 copies."""
    u1 = rtmp.tile([128, w], F16, tag="u1", name="u1")
    u2 = rtmp.tile([128, w], F16, tag="u2", name="u2")
    nc.vector.tensor_mul(u1, ps, cs)         # [te*c ; to*s]
    nc.vector.tensor_mul(u2, ps, cs_w)       # [te*s ; to*c] -- frees psum
    b1 = rtmp.tile([64, w], F16, tag="b1", name="b1")
    b2 = rtmp.tile([64, w], F16, tag="b2", name="b2")
    nc.vector.tensor_copy(b1, u1[64:128, :])
    nc.vector.tensor_copy(b2, u2[64:128, :])
    nc.vector.tensor_sub(out_sl[0:64, :], u1[0:64, :], b1)
    nc.vector.tensor_add(out_sl[64:128, :], u2[0:64, :], b2)


def build_nc(causal, shared, s=S, d=D, qc_w=512):
    assert HL == 2
    ndt = d // 128          # contraction tiles over model dim
    nkt = s // 128          # kpos tiles
    nsc = s // qc_w         # seq/q chunks
    nst = qc_w // 128       # kpos tiles per q-chunk band
    rw = s // NC
    sc_w = qc_w
    nj = qc_w // rw         # dest cores covered by one q chunk

    nc = bacc.Bacc("TRN2", target_bir_lowering=False, debug=False, num_devices=NC)

    xt = nc.dram_tensor("xt", [d, s], F16, kind="ExternalInput").ap()
    wqkv = nc.dram_tensor("wqkv", [d, 3 * CW], F16, kind="ExternalInput").ap()
    wo = nc.dram_tensor("wo", [d, d], BF16, kind="ExternalInput").ap()
    cq = nc.dram_tensor("cq", [128, s], F16, kind="ExternalInput").ap()
    if not shared:
        sq = nc.dram_tensor("sq", [128, s], F16, kind="ExternalInput").ap()
    cksk = nc.dram_tensor("cksk", [128, s], F16, kind="ExternalInput").ap()
    if causal:
        # staircase: diagonal pattern m is emb[:, 512-128m : 1024-128m]
        emb = nc.dram_tensor("em", [128, qc_w + nst * 128], BF16,
                             kind="ExternalInput").ap()
    else:
        emb = nc.dram_tensor("em", [s, s], BF16, kind="ExternalInput").ap()
    out = nc.dram_tensor("out", [rw, d], F32, kind="ExternalOutput").ap()

    import contextlib

    with tile.TileContext(nc, num_cores=NC) as tc:
        with contextlib.ExitStack() as top:
            qkv = top.enter_context(tc.tile_pool(name="qkv", bufs=1))
            qT_s = qkv.tile([128, HL, s], F16)
            kT_s = qkv.tile([128, HL, s], F16)
            v_s = qkv.tile([128, nkt, CW], BF16)
            ones_s = qkv.tile([128, 128], BF16)
            nc.vector.memset(ones_s, 1.0)
            dram = top.enter_context(tc.tile_pool(name="dram", bufs=1, space="DRAM"))
            a2a_in = [dram.tile([NC, HD, rw], BF16, name=f"a2ain{_h}") for _h in range(HL)]
            a2a_out = [dram.tile([NC, HD, rw], BF16, name=f"a2aout{_h}") for _h in range(HL)]

            # tiny warm-up collective: absorbs first-collective setup cost
            warm_i = dram.tile([NC, 1, 64], BF16, name="warm_i")
            warm_o = dram.tile([NC, 1, 64], BF16, name="warm_o")
            wz = qkv.tile([1, NC * 64], BF16)
            nc.vector.memset(wz, 0.0)
            nc.scalar.dma_start(warm_i.rearrange("a b c -> b (a c)"), wz)
            nc.gpsimd.collective_compute(
                "AllToAll",
                mybir.AluOpType.bypass,
                replica_groups=[list(range(NC))],
                ins=[warm_i.opt()],
                outs=[warm_o.opt()],
            )

            consts = top.enter_context(tc.tile_pool(name="consts", bufs=1))
            cq_s = consts.tile([128, s], F16)     # q [c;s] (scaled)
            cqw_s = consts.tile([128, s], F16)    # swapped [s;c]
            if not shared:
                sq_s = consts.tile([128, s], F16)
                sqw_s = consts.tile([128, s], F16)
            else:
                sq_s, sqw_s = cq_s, cqw_s
            ck_s = consts.tile([128, s], F16)     # k [c;s]
            ckw_s = consts.tile([128, s], F16)
            wqkv_sb = consts.tile([128, ndt, 3 * CW], F16)
            em_s = None
            if causal:
                em_s = consts.tile([128, qc_w + nst * 128], BF16)

            wop = top.enter_context(tc.tile_pool(name="wop", bufs=1))
            wo_sb = wop.tile([128, ndt, d], BF16)
            wo_r = wo.rearrange("(kt p) n -> kt p n", p=128)

            p4 = top.enter_context(tc.tile_pool(name="p4", bufs=1))
            lhs1_sb = p4.tile([128, NC, rw], BF16)
            lhs2_sb = p4.tile([128, NC, rw], BF16)
            o_acc = p4.tile([128, rw // 128, d], BF16)

            xch = top.enter_context(tc.tile_pool(name="xch", bufs=10))
            rtmp = top.enter_context(tc.tile_pool(name="rtmp", bufs=1))
            ep = top.enter_context(tc.tile_pool(name="ep", bufs=3))
            rbp = top.enter_context(tc.tile_pool(name="rbp", bufs=2))
            attp = top.enter_context(tc.tile_pool(name="attp", bufs=2))
            accp = top.enter_context(tc.tile_pool(name="accp", bufs=2))
            outp = top.enter_context(tc.tile_pool(name="outp", bufs=2))
            emp = None
            if not causal:
                emp = top.enter_context(tc.tile_pool(name="emp", bufs=4))

            psqk = top.enter_context(tc.tile_pool(name="psqk", bufs=3, space="PSUM"))
            psv = top.enter_context(tc.tile_pool(name="psv", bufs=2, space="PSUM"))
            pss = top.enter_context(tc.tile_pool(name="pss", bufs=2, space="PSUM"))
            pso = top.enter_context(tc.tile_pool(name="pso", bufs=1, space="PSUM"))

            wqkv_p = wqkv.rearrange("(dt p) c -> p dt c", p=128)
            xt_p = xt.rearrange("(dt p) z -> p dt z", p=128)

            # tables + mask on the scalar DMA queue: the sync queue stays a
            # pure x/weight stream so the first matmuls start early
            nc.scalar.dma_start(cq_s, cq)
            if not shared:
                nc.scalar.dma_start(sq_s, sq)
            nc.scalar.dma_start(ck_s, cksk)
            if causal:
                nc.scalar.dma_start(em_s, emb)

            def attn_units(h, qc, dst):
                """Generator: one attention tile per step; PV deferred one
                tile so exp latency hides behind the next scores matmul."""
                qsl = slice(qc * qc_w, (qc + 1) * qc_w)
                n_kt = nst * (qc + 1) if causal else nkt
                o_ps = pso.tile([128, qc_w], F32, tag="pso", name=f"ops{h}_{qc}")
                acc = accp.tile([128, qc_w], BF16, tag="acc", name=f"acc{h}_{qc}")
                prev = None
                for kt in range(n_kt):
                    s_ps = pss.tile([128, qc_w], F32, tag="pss", name=f"sps{h}_{qc}_{kt}")
                    nc.tensor.matmul(
                        s_ps,
                        lhsT=kT_s[:, h, kt * 128 : (kt + 1) * 128],
                        rhs=qT_s[:, h, qsl],
                        start=True,
                        stop=True,
                    )
                    if prev is not None:
                        nc.tensor.matmul(
                            o_ps,
                            lhsT=v_s[:, prev[1], HD * h : HD * (h + 1)],
                            rhs=prev[0],
                            start=(prev[1] == 0),
                            stop=False,
                        )
                    e = ep.tile([128, qc_w], BF16, tag="e", name=f"e{h}_{qc}_{kt}")
                    nc.scalar.activation(e, s_ps, AF.Exp)
                    if causal:
                        m = kt - nst * qc
                        if m >= 0:
                            off = qc_w - 128 * m
                            nc.vector.tensor_mul(e, e, em_s[:, off : off + qc_w])
                    else:
                        emt = emp.tile([128, qc_w], BF16, tag="em", name=f"emt{h}_{qc}_{kt}")
                        nc.sync.dma_start(emt, emb[kt * 128 : (kt + 1) * 128, qsl])
                        nc.vector.tensor_mul(e, e, emt)
                    if kt == 0:
                        nc.vector.tensor_copy(acc, e)
                    else:
                        nc.vector.tensor_add(acc, acc, e)
                    prev = (e, kt)
                    yield
                nc.tensor.matmul(
                    o_ps,
                    lhsT=v_s[:, prev[1], HD * h : HD * (h + 1)],
                    rhs=prev[0],
                    start=(prev[1] == 0),
                    stop=True,
                )
                # denominator, pre-broadcast across partitions by a [128,128]
                # ones stationary against the bf16 accumulator
                d_ps = pss.tile([128, qc_w], F32, tag="pss", name=f"dps{h}_{qc}")
                nc.tensor.matmul(d_ps, lhsT=ones_s, rhs=acc, start=True, stop=True)
                rec = rbp.tile([128, qc_w], F32, tag="rb", name=f"rb{h}_{qc}")
                nc.vector.reciprocal_approx_fast(rec, d_ps)
                att = attp.tile([128, qc_w], BF16, tag="att", name=f"att{h}_{qc}")
                nc.vector.tensor_mul(att, o_ps, rec)
                for j in range(nj):
                    nc.sync.dma_start(
                        dst[qc * nj + j, :, :],
                        att[:, j * rw : (j + 1) * rw],
                    )
                yield

            def step(g):
                if g is None:
                    return False
                try:
                    next(g)
                    return True
                except StopIteration:
                    return False

            def drain(g):
                while step(g):
                    pass

            # ---------------- phase 1 with interleaved attention ----------------
            for sc in range(nsc):
                scs = slice(sc * sc_w, (sc + 1) * sc_w)
                xps = []
                for dp in range(ndt // 2):
                    if sc == 0:
                        nc.sync.dma_start(
                            wqkv_sb[:, 2 * dp : 2 * dp + 2, :],
                            wqkv_p[:, 2 * dp : 2 * dp + 2, :],
                        )
                    t = xch.tile([128, 2, sc_w], F16, tag="xch", name=f"xch{sc}_{dp}")
                    nc.sync.dma_start(t, xt_p[:, 2 * dp : 2 * dp + 2, scs])
                    xps.append(t)
                xts = [xps[dt // 2][:, dt % 2, :] for dt in range(ndt)]
                if sc == 0:
                    pairs = [(cq_s, cqw_s), (ck_s, ckw_s)]
                    if not shared:
                        pairs.insert(1, (sq_s, sqw_s))
                    for src_t, dst_t in pairs:
                        nc.vector.tensor_copy(dst_t[0:64, :], src_t[64:128, :])
                        nc.vector.tensor_copy(dst_t[64:128, :], src_t[0:64, :])

                # head-1 attention lags two chunks: q-chunks 2 and 3 stay
                # for the tail, where they keep the PE busy and the HAM
                # clock warm while the first AllToAll absorbs core skew
                gen1 = attn_units(1, sc - 2, a2a_in[1]) if (causal and sc >= 2) else None
                q_ps = [psqk.tile([128, sc_w], F32, tag="psqk", name=f"qps{sc}_{_h}")
                        for _h in range(HL)]
                for dt in range(ndt):
                    fl = dict(start=(dt == 0), stop=(dt == ndt - 1))
                    for h in range(HL):
                        nc.tensor.matmul(
                            q_ps[h],
                            lhsT=wqkv_sb[:, dt, HD * h : HD * (h + 1)],
                            rhs=xts[dt],
                            **fl,
                        )
                    if dt >= 1:
                        step(gen1)
                drain(gen1)
                _rope_drain(nc, rtmp, q_ps[0], qT_s[:, 0, scs],
                            cq_s[:, scs], cqw_s[:, scs], sc_w)
                _rope_drain(nc, rtmp, q_ps[1], qT_s[:, 1, scs],
                            sq_s[:, scs], sqw_s[:, scs], sc_w)

                # head-0 attention for THIS q chunk: off-band tiles ride the
                # k chains, diagonal-band tiles ride the v chains
                gen0 = attn_units(0, sc, a2a_in[0]) if causal else None
                off_band = nst * sc
                emitted = 0
                # k0 chain, then k1 chain (k1's psum ring slot frees after
                # rope_q0, long done by then -- no PE stall)
                k1_ps = None
                for h in range(HL):
                    k_ps = psqk.tile([128, sc_w], F32, tag="psqk", name=f"kps{sc}_{h}")
                    for dt in range(ndt):
                        nc.tensor.matmul(
                            k_ps,
                            lhsT=wqkv_sb[:, dt, CW + HD * h : CW + HD * (h + 1)],
                            rhs=xts[dt],
                            start=(dt == 0),
                            stop=(dt == ndt - 1),
                        )
                        if dt >= 2 and emitted < off_band:
                            if step(gen0):
                                emitted += 1
                    if h == 0:
                        _rope_drain(nc, rtmp, k_ps, kT_s[:, 0, scs],
                                    ck_s[:, scs], ckw_s[:, scs], sc_w)
                    else:
                        k1_ps = k_ps
                # v chains; rope_k1 + v copies interleave on DVE; the
                # diagonal-band attention tiles follow their v copies
                for st in range(nst):
                    v_ps = psv.tile([128, CW], F32, tag="psv", name=f"vps{sc}_{st}")
                    for dt in range(ndt):
                        nc.tensor.matmul(
                            v_ps,
                            lhsT=xts[dt][:, st * 128 : (st + 1) * 128],
                            rhs=wqkv_sb[:, dt, 2 * CW : 3 * CW],
                            start=(dt == 0),
                            stop=(dt == ndt - 1),
                        )
                    nc.vector.tensor_copy(v_s[:, sc * nst + st, :], v_ps)
                    if st == 0:
                        _rope_drain(nc, rtmp, k1_ps, kT_s[:, 1, scs],
                                    ck_s[:, scs], ckw_s[:, scs], sc_w)
                    if st >= 1:
                        step(gen0)
                drain(gen0)
                if causal and sc == nsc - 1:
                    nc.gpsimd.collective_compute(
                        "AllToAll",
                        mybir.AluOpType.bypass,
                        replica_groups=[list(range(NC))],
                        ins=[a2a_in[0].opt()],
                        outs=[a2a_out[0].opt()],
                    )

                # wo prefetch on gpsimd (idle but for collective triggers)
                for kt in range(nst):
                    nc.gpsimd.dma_start(wo_sb[:, nst * sc + kt, :], wo_r[nst * sc + kt])
                if sc == 0:
                    # second tiny collective: the first real AllToAll then
                    # runs on a fully warmed CC path
                    nc.gpsimd.collective_compute(
                        "AllToAll",
                        mybir.AluOpType.bypass,
                        replica_groups=[list(range(NC))],
                        ins=[warm_i.opt()],
                        outs=[warm_o.opt()],
                    )

            if causal:
                drain(attn_units(1, nsc - 2, a2a_in[1]))
                drain(attn_units(1, nsc - 1, a2a_in[1]))
            else:
                for qc in range(nsc):
                    drain(attn_units(0, qc, a2a_in[0]))
                nc.gpsimd.collective_compute(
                    "AllToAll",
                    mybir.AluOpType.bypass,
                    replica_groups=[list(range(NC))],
                    ins=[a2a_in[0].opt()],
                    outs=[a2a_out[0].opt()],
                )
                for qc in range(nsc):
                    drain(attn_units(1, qc, a2a_in[1]))
            nc.gpsimd.collective_compute(
                "AllToAll",
                mybir.AluOpType.bypass,
                replica_groups=[list(range(NC))],
                ins=[a2a_in[1].opt()],
                outs=[a2a_out[1].opt()],
            )

            # HAM keep-alive through the collective barrier: cores that
            # finish attention early idle here, drop to the 1.2 GHz cold
            # clock, and run the output projection at half speed, ending
            # last despite arriving first. A serialized copy chain on the
            # ScalarE (idle after the last exp; shares no SBUF port pair
            # with DVE) paces one tiny matmul every ~2us so the PE's
            # activity monitor keeps the clock warm. On the straggler core
            # (no idle) this costs ~1us total.
            scr = p4.tile([128, 8, CW], BF16)
            for i in range(10):
                nc.scalar.activation(scr, v_s[:, i % 8 : i % 8 + 8, :], AF.Copy)
                kap = pso.tile([128, 128], F32, tag="pso", name=f"kap{i}")
                nc.tensor.matmul(
                    kap, lhsT=ones_s, rhs=scr[:, 0, 0:128], start=True, stop=True
                )

            # ---------------- output projection ----------------
            def wo_part(lhs_sb, col, final):
                for mt in range(rw // 128):
                    for nk in range(d // 512):
                        nsl = slice(nk * 512, (nk + 1) * 512)
                        w_ps = pss.tile([128, 512], F32, tag="pss", name=f"wps{col}_{mt}_{nk}")
                        for j in range(NC):
                            nc.tensor.matmul(
                                w_ps,
                                lhsT=lhs_sb[:, j, mt * 128 : (mt + 1) * 128],
                                rhs=wo_sb[:, 2 * j + col, nsl],
                                start=(j == 0),
                                stop=(j == NC - 1),
                            )
                        if not final:
                            nc.vector.tensor_copy(o_acc[:, mt, nsl], w_ps)
                        else:
                            o_sb = outp.tile([128, 512], F32, tag="osb", name=f"osb{mt}_{nk}")
                            nc.vector.tensor_add(o_sb, o_acc[:, mt, nsl], w_ps)
                            nc.sync.dma_start(
                                out[mt * 128 : (mt + 1) * 128, nsl], o_sb
                            )

            nc.sync.dma_start(lhs1_sb, a2a_out[0].rearrange("j p q -> p j q"))
            wo_part(lhs1_sb, 0, final=False)
            nc.sync.dma_start(lhs2_sb, a2a_out[1].rearrange("j p q -> p j q"))
            wo_part(lhs2_sb, 1, final=True)

    nc.compile()
    return nc


def host_prep(inputs, s=S, d=D, qc_w=512):
    f16 = np.float16
    x = np.ascontiguousarray(np.asarray(inputs["x"], dtype=np.float32)[0])
    wq = np.asarray(inputs["wq"], dtype=np.float32)
    wk = np.asarray(inputs["wk"], dtype=np.float32)
    wv = np.asarray(inputs["wv"], dtype=np.float32)
    wo = np.asarray(inputs["wo"], dtype=np.float32)
    ss = np.asarray(inputs["seq_scale"], dtype=np.float32).reshape(H)
    cos = np.asarray(inputs["freqs_cos"], dtype=np.float32)
    sin = np.asarray(inputs["freqs_sin"], dtype=np.float32)
    mask = np.asarray(inputs["mask"], dtype=np.float32)[0, 0]
    sll = np.asarray(inputs["section_log_len"], dtype=np.float32).reshape(s)

    nst = qc_w // 128
    zero = mask == 0.0
    causal = bool(
        np.array_equal(zero, np.tril(np.ones((s, s), bool)))
        and np.all(mask[~zero] <= -1e8)
    )
    shared = bool(np.all(ss == ss[0]))

    if causal:
        # staircase base [128, qc_w + nst*128]: diagonal pattern m (kpos
        # tile kt = nst*qc + m vs q chunk qc) is emb[:, qc_w-128m :
        # 2*qc_w-128m], i.e. emb[dk, c] = 1 iff (dk + 128m) <= dq with
        # dq = c - (qc_w - 128m)  <=>  dk <= c - qc_w
        w_em = qc_w + nst * 128
        ccol = np.arange(w_em)[None, :]
        crow = np.arange(128)[:, None]
        em_in = np.ascontiguousarray(
            (crow <= ccol - qc_w).astype(np.float32).astype(bf16)
        )
    else:
        em_in = np.ascontiguousarray(np.exp(np.minimum(mask, 0.0)).T.astype(bf16))

    perm = np.concatenate([np.arange(0, HD, 2), np.arange(1, HD, 2)])
    xt = np.ascontiguousarray(x.T.astype(f16))
    scale = sll / np.sqrt(HD)
    cksk = np.ascontiguousarray(np.concatenate([cos.T, sin.T], axis=0).astype(f16))
    wo_b = np.ascontiguousarray(wo.astype(bf16))

    in_maps = []
    for i in range(NC):
        wq_s = np.concatenate(
            [wq[:, CW * i + HD * h : CW * i + HD * (h + 1)][:, perm] for h in range(HL)],
            axis=1,
        )
        wk_s = np.concatenate(
            [wk[:, CW * i + HD * h : CW * i + HD * (h + 1)][:, perm] for h in range(HL)],
            axis=1,
        )
        wv_s = wv[:, CW * i : CW * (i + 1)]
        # per-head packed [cos; sin] scaled tables (one shared table when
        # seq_scale is uniform)
        cqt = np.concatenate(
            [cos.T * (scale * ss[HL * i])[None, :],
             sin.T * (scale * ss[HL * i])[None, :]], axis=0
        )
        m = {
            "xt": xt,
            "wqkv": np.ascontiguousarray(
                np.concatenate([wq_s, wk_s, wv_s], axis=1).astype(f16)
            ),
            "wo": wo_b,
            "cq": np.ascontiguousarray(cqt.astype(f16)),
            "cksk": cksk,
            "em": em_in,
        }
        if not shared:
            sqt = np.concatenate(
                [cos.T * (scale * ss[HL * i + 1])[None, :],
                 sin.T * (scale * ss[HL * i + 1])[None, :]], axis=0
            )
            m["sq"] = np.ascontiguousarray(sqt.astype(f16))
        in_maps.append(m)
    return in_maps, causal, shared


_NC_CACHE = {}


def _get_nc(causal, shared):
    key = (causal, shared)
    if key not in _NC_CACHE:
        _NC_CACHE[key] = build_nc(causal, shared)
    return _NC_CACHE[key]


def kernel(**inputs) -> np.ndarray:
    in_maps, causal, shared = host_prep(inputs)
    nc = _get_nc(causal, shared)
    res = run_bass_kernel_spmd(nc, in_maps, core_ids=list(range(NC)))
    rows = [res.results[i]["out"] for i in range(NC)]
    return np.concatenate(rows, axis=0)[None].astype(np.float32)


# revision 14
# speedup vs baseline: 1.0367x; 1.0367x over previous
"""8-core Trainium2 Bass kernel for nn_Attention_54778012893378.

Tensor-parallel over heads (2 heads/core). v4: fully 16-bit PE datapath
(fp16 for x/weights/q/k/scores, bf16 where exp range demands it), with
attention emitted tile-by-tile INSIDE the projection dt-loops. The exp on
ScalarE (685ns/tile) outweighs the PE work per attention tile (426ns), so
attention alone stalls the in-order PE queue; interleaving projection
matmuls between attention tiles keeps the PE saturated and the HAM clock
gate at the full 2.4 GHz (idle gaps re-throttle the PE to 1.2 GHz).

Schedule per seq chunk sc (512 positions):
  q-chain (x@wq, both heads) with head-1/q-chunk sc-1 attention tiles
  between dt steps; rope_q on DVE; k0-chain then k1-chain (k1 waits
  rope_q0's psum ring slot, which is free by then) with head-0/q-chunk sc
  off-band attention tiles between dt steps; rope_k0; v st-chains with
  head-0 diagonal-band tiles between them (rope_k1 + v copies interleaved
  on DVE so nothing stalls).
Head-0's AllToAll fires inside the last chunk; head-1/q-chunk 3 is the only
post-loop attention, overlapping that collective; the two wo halves overlap
head-1's AllToAll. wo streams on gpsimd early (the collective triggers
queue behind it harmlessly); a second tiny warm-up collective ensures the
first real AllToAll runs on a warmed CC path. Scores are scaled via the
per-query factor folded into q's rope tables (single shared table when
seq_scale is uniform); the causal mask is 4 shifted slices of one bf16
staircase tile, applied as a multiply after exp; the softmax denominator
accumulates in bf16 on DVE and broadcasts via one ones-matmul.
"""

import numpy as np
import ml_dtypes

import concourse.bass as bass
import concourse.bacc as bacc
import concourse.tile as tile
import concourse.mybir as mybir
from concourse.bass_utils import run_bass_kernel_spmd

F32 = mybir.dt.float32
F32R = mybir.dt.float32r
F16 = mybir.dt.float16
BF16 = mybir.dt.bfloat16
AF = mybir.ActivationFunctionType
bf16 = ml_dtypes.bfloat16

# problem dims (hardcoded per spec)
S, D, H, HD, NC = 2048, 2048, 16, 128, 8
HL = H // NC            # local heads per core
CW = HL * HD            # per-core head-column width
RW = S // NC            # per-core output row width


def _rope_drain(nc, rtmp, ps, out_sl, cs, cs_w, w):
    """Full-width rope from psum [te;to]: oe = te*c - to*s ; oo = te*s + to*c.
    cs = [c;s] packed [128, w]; cs_w = [s;c] (swapped). Mixed psum+sbuf
    operands may differ in base partition; both-sbuf operands may not, so
    the upper halves are staged through base-0 copies."""
    u1 = rtmp.tile([128, w], F16, tag="u1", name="u1")
    u2 = rtmp.tile([128, w], F16, tag="u2", name="u2")
    nc.vector.tensor_mul(u1, ps, cs)         # [te*c ; to*s]
    nc.vector.tensor_mul(u2, ps, cs_w)       # [te*s ; to*c] -- frees psum
    b1 = rtmp.tile([64, w], F16, tag="b1", name="b1")
    b2 = rtmp.tile([64, w], F16, tag="b2", name="b2")
    nc.vector.tensor_copy(b1, u1[64:128, :])
    nc.vector.tensor_copy(b2, u2[64:128, :])
    nc.vector.tensor_sub(out_sl[0:64, :], u1[0:64, :], b1)
    nc.vector.tensor_add(out_sl[64:128, :], u2[0:64, :], b2)


def build_nc(causal, shared, s=S, d=D, qc_w=512):
    assert HL == 2
    ndt = d // 128          # contraction tiles over model dim
    nkt = s // 128          # kpos tiles
    nsc = s // qc_w         # seq/q chunks
    nst = qc_w // 128       # kpos tiles per q-chunk band
    rw = s // NC
    sc_w = qc_w
    nj = qc_w // rw         # dest cores covered by one q chunk

    nc = bacc.Bacc("TRN2", target_bir_lowering=False, debug=False, num_devices=NC)

    xt = nc.dram_tensor("xt", [d, s], F16, kind="ExternalInput").ap()
    wqkv = nc.dram_tensor("wqkv", [d, 3 * CW], F16, kind="ExternalInput").ap()
    wo = nc.dram_tensor("wo", [d, d], BF16, kind="ExternalInput").ap()
    cq = nc.dram_tensor("cq", [128, s], F16, kind="ExternalInput").ap()
    if not shared:
        sq = nc.dram_tensor("sq", [128, s], F16, kind="ExternalInput").ap()
    cksk = nc.dram_tensor("cksk", [128, s], F16, kind="ExternalInput").ap()
    if causal:
        # staircase: diagonal pattern m is emb[:, 512-128m : 1024-128m]
        emb = nc.dram_tensor("em", [128, qc_w + nst * 128], BF16,
                             kind="ExternalInput").ap()
    else:
        emb = nc.dram_tensor("em", [s, s], BF16, kind="ExternalInput").ap()
    out = nc.dram_tensor("out", [rw, d], F32, kind="ExternalOutput").ap()

    import contextlib

    with tile.TileContext(nc, num_cores=NC) as tc:
        with contextlib.ExitStack() as top:
            qkv = top.enter_context(tc.tile_pool(name="qkv", bufs=1))
            qT_s = qkv.tile([128, HL, s], F16)
            kT_s = qkv.tile([128, HL, s], F16)
            v_s = qkv.tile([128, nkt, CW], BF16)
            ones_s = qkv.tile([128, 128], BF16)
            nc.vector.memset(ones_s, 1.0)
            dram = top.enter_context(tc.tile_pool(name="dram", bufs=1, space="DRAM"))
            a2a_in = [dram.tile([NC, HD, rw], BF16, name=f"a2ain{_h}") for _h in range(HL)]
            a2a_out = [dram.tile([NC, HD, rw], BF16, name=f"a2aout{_h}") for _h in range(HL)]

            # tiny warm-up collective: absorbs first-collective setup cost
            warm_i = dram.tile([NC, 1, 64], BF16, name="warm_i")
            warm_o = dram.tile([NC, 1, 64], BF16, name="warm_o")
            wz = qkv.tile([1, NC * 64], BF16)
            nc.vector.memset(wz, 0.0)
            nc.scalar.dma_start(warm_i.rearrange("a b c -> b (a c)"), wz)
            nc.gpsimd.collective_compute(
                "AllToAll",
                mybir.AluOpType.bypass,
                replica_groups=[list(range(NC))],
                ins=[warm_i.opt()],
                outs=[warm_o.opt()],
            )

            consts = top.enter_context(tc.tile_pool(name="consts", bufs=1))
            cq_s = consts.tile([128, s], F16)     # q [c;s] (scaled)
            cqw_s = consts.tile([128, s], F16)    # swapped [s;c]
            if not shared:
                sq_s = consts.tile([128, s], F16)
                sqw_s = consts.tile([128, s], F16)
            else:
                sq_s, sqw_s = cq_s, cqw_s
            ck_s = consts.tile([128, s], F16)     # k [c;s]
            ckw_s = consts.tile([128, s], F16)
            wqkv_sb = consts.tile([128, ndt, 3 * CW], F16)
            em_s = None
            if causal:
                em_s = consts.tile([128, qc_w + nst * 128], BF16)

            wop = top.enter_context(tc.tile_pool(name="wop", bufs=1))
            wo_sb = wop.tile([128, ndt, d], BF16)
            wo_r = wo.rearrange("(kt p) n -> kt p n", p=128)

            p4 = top.enter_context(tc.tile_pool(name="p4", bufs=1))
            lhs1_sb = p4.tile([128, NC, rw], BF16)
            lhs2_sb = p4.tile([128, NC, rw], BF16)
            o_acc = p4.tile([128, rw // 128, d], BF16)

            xch = top.enter_context(tc.tile_pool(name="xch", bufs=10))
            rtmp = top.enter_context(tc.tile_pool(name="rtmp", bufs=1))
            ep = top.enter_context(tc.tile_pool(name="ep", bufs=3))
            rbp = top.enter_context(tc.tile_pool(name="rbp", bufs=2))
            attp = top.enter_context(tc.tile_pool(name="attp", bufs=2))
            accp = top.enter_context(tc.tile_pool(name="accp", bufs=2))
            outp = top.enter_context(tc.tile_pool(name="outp", bufs=2))
            emp = None
            if not causal:
                emp = top.enter_context(tc.tile_pool(name="emp", bufs=4))

            psqk = top.enter_context(tc.tile_pool(name="psqk", bufs=3, space="PSUM"))
            psv = top.enter_context(tc.tile_pool(name="psv", bufs=2, space="PSUM"))
            pss = top.enter_context(tc.tile_pool(name="pss", bufs=2, space="PSUM"))
            pso = top.enter_context(tc.tile_pool(name="pso", bufs=1, space="PSUM"))

            wqkv_p = wqkv.rearrange("(dt p) c -> p dt c", p=128)
            xt_p = xt.rearrange("(dt p) z -> p dt z", p=128)

            # mask pattern on the scalar DMA queue (tiny); the big rope
            # tables go at the tail of the sc0 x/weight stream -- needed
            # only by rope at ~40us, they must not steal early HBM from x
            if causal:
                nc.scalar.dma_start(em_s, emb)

            def attn_units(h, qc, dst):
                """Generator: one attention tile per step; PV deferred one
                tile so exp latency hides behind the next scores matmul."""
                qsl = slice(qc * qc_w, (qc + 1) * qc_w)
                n_kt = nst * (qc + 1) if causal else nkt
                o_ps = pso.tile([128, qc_w], F32, tag="pso", name=f"ops{h}_{qc}")
                acc = accp.tile([128, qc_w], BF16, tag="acc", name=f"acc{h}_{qc}")
                prev = None
                for kt in range(n_kt):
                    s_ps = pss.tile([128, qc_w], F32, tag="pss", name=f"sps{h}_{qc}_{kt}")
                    nc.tensor.matmul(
                        s_ps,
                        lhsT=kT_s[:, h, kt * 128 : (kt + 1) * 128],
                        rhs=qT_s[:, h, qsl],
                        start=True,
                        stop=True,
                    )
                    if prev is not None:
                        nc.tensor.matmul(
                            o_ps,
                            lhsT=v_s[:, prev[1], HD * h : HD * (h + 1)],
                            rhs=prev[0],
                            start=(prev[1] == 0),
                            stop=False,
                        )
                    e = ep.tile([128, qc_w], BF16, tag="e", name=f"e{h}_{qc}_{kt}")
                    nc.scalar.activation(e, s_ps, AF.Exp)
                    if causal:
                        m = kt - nst * qc
                        if m >= 0:
                            off = qc_w - 128 * m
                            nc.vector.tensor_mul(e, e, em_s[:, off : off + qc_w])
                    else:
                        emt = emp.tile([128, qc_w], BF16, tag="em", name=f"emt{h}_{qc}_{kt}")
                        nc.sync.dma_start(emt, emb[kt * 128 : (kt + 1) * 128, qsl])
                        nc.vector.tensor_mul(e, e, emt)
                    if kt == 0:
                        nc.vector.tensor_copy(acc, e)
                    else:
                        nc.vector.tensor_add(acc, acc, e)
                    prev = (e, kt)
                    yield
                nc.tensor.matmul(
                    o_ps,
                    lhsT=v_s[:, prev[1], HD * h : HD * (h + 1)],
                    rhs=prev[0],
                    start=(prev[1] == 0),
                    stop=True,
                )
                # denominator, pre-broadcast across partitions by a [128,128]
                # ones stationary against the bf16 accumulator
                d_ps = pss.tile([128, qc_w], F32, tag="pss", name=f"dps{h}_{qc}")
                nc.tensor.matmul(d_ps, lhsT=ones_s, rhs=acc, start=True, stop=True)
                rec = rbp.tile([128, qc_w], F32, tag="rb", name=f"rb{h}_{qc}")
                nc.vector.reciprocal_approx_fast(rec, d_ps)
                att = attp.tile([128, qc_w], BF16, tag="att", name=f"att{h}_{qc}")
                nc.vector.tensor_mul(att, o_ps, rec)
                for j in range(nj):
                    nc.sync.dma_start(
                        dst[qc * nj + j, :, :],
                        att[:, j * rw : (j + 1) * rw],
                    )
                yield

            def step(g):
                if g is None:
                    return False
                try:
                    next(g)
                    return True
                except StopIteration:
                    return False

            def drain(g):
                while step(g):
                    pass

            # ---------------- phase 1 with interleaved attention ----------------
            for sc in range(nsc):
                scs = slice(sc * sc_w, (sc + 1) * sc_w)
                xps = []
                for dp in range(ndt // 2):
                    if sc == 0:
                        nc.sync.dma_start(
                            wqkv_sb[:, 2 * dp : 2 * dp + 2, :],
                            wqkv_p[:, 2 * dp : 2 * dp + 2, :],
                        )
                    t = xch.tile([128, 2, sc_w], F16, tag="xch", name=f"xch{sc}_{dp}")
                    nc.sync.dma_start(t, xt_p[:, 2 * dp : 2 * dp + 2, scs])
                    xps.append(t)
                xts = [xps[dt // 2][:, dt % 2, :] for dt in range(ndt)]
                if sc == 0:
                    nc.sync.dma_start(cq_s, cq)
                    if not shared:
                        nc.sync.dma_start(sq_s, sq)
                    nc.sync.dma_start(ck_s, cksk)
                    pairs = [(cq_s, cqw_s), (ck_s, ckw_s)]
                    if not shared:
                        pairs.insert(1, (sq_s, sqw_s))
                    for src_t, dst_t in pairs:
                        nc.vector.tensor_copy(dst_t[0:64, :], src_t[64:128, :])
                        nc.vector.tensor_copy(dst_t[64:128, :], src_t[0:64, :])

                # head-1 attention lags two chunks: q-chunks 2 and 3 stay
                # for the tail, where they keep the PE busy and the HAM
                # clock warm while the first AllToAll absorbs core skew
                gen1 = attn_units(1, sc - 2, a2a_in[1]) if (causal and sc >= 2) else None
                q_ps = [psqk.tile([128, sc_w], F32, tag="psqk", name=f"qps{sc}_{_h}")
                        for _h in range(HL)]
                for dt in range(ndt):
                    fl = dict(start=(dt == 0), stop=(dt == ndt - 1))
                    for h in range(HL):
                        nc.tensor.matmul(
                            q_ps[h],
                            lhsT=wqkv_sb[:, dt, HD * h : HD * (h + 1)],
                            rhs=xts[dt],
                            **fl,
                        )
                    if dt >= 1:
                        step(gen1)
                drain(gen1)
                _rope_drain(nc, rtmp, q_ps[0], qT_s[:, 0, scs],
                            cq_s[:, scs], cqw_s[:, scs], sc_w)
                _rope_drain(nc, rtmp, q_ps[1], qT_s[:, 1, scs],
                            sq_s[:, scs], sqw_s[:, scs], sc_w)

                # head-0 attention for THIS q chunk: off-band tiles ride the
                # k chains, diagonal-band tiles ride the v chains
                gen0 = attn_units(0, sc, a2a_in[0]) if causal else None
                off_band = nst * sc
                emitted = 0
                # k0 chain, then k1 chain (k1's psum ring slot frees after
                # rope_q0, long done by then -- no PE stall)
                k1_ps = None
                for h in range(HL):
                    k_ps = psqk.tile([128, sc_w], F32, tag="psqk", name=f"kps{sc}_{h}")
                    for dt in range(ndt):
                        nc.tensor.matmul(
                            k_ps,
                            lhsT=wqkv_sb[:, dt, CW + HD * h : CW + HD * (h + 1)],
                            rhs=xts[dt],
                            start=(dt == 0),
                            stop=(dt == ndt - 1),
                        )
                        if dt >= 2 and emitted < off_band:
                            if step(gen0):
                                emitted += 1
                    if h == 0:
                        _rope_drain(nc, rtmp, k_ps, kT_s[:, 0, scs],
                                    ck_s[:, scs], ckw_s[:, scs], sc_w)
                    else:
                        k1_ps = k_ps
                # v chains; rope_k1 + v copies interleave on DVE; the
                # diagonal-band attention tiles follow their v copies
                for st in range(nst):
                    v_ps = psv.tile([128, CW], F32, tag="psv", name=f"vps{sc}_{st}")
                    for dt in range(ndt):
                        nc.tensor.matmul(
                            v_ps,
                            lhsT=xts[dt][:, st * 128 : (st + 1) * 128],
                            rhs=wqkv_sb[:, dt, 2 * CW : 3 * CW],
                            start=(dt == 0),
                            stop=(dt == ndt - 1),
                        )
                    nc.vector.tensor_copy(v_s[:, sc * nst + st, :], v_ps)
                    if st == 0:
                        _rope_drain(nc, rtmp, k1_ps, kT_s[:, 1, scs],
                                    ck_s[:, scs], ckw_s[:, scs], sc_w)
                    if st >= 1:
                        step(gen0)
                drain(gen0)
                if causal and sc == nsc - 1:
                    nc.gpsimd.collective_compute(
                        "AllToAll",
                        mybir.AluOpType.bypass,
                        replica_groups=[list(range(NC))],
                        ins=[a2a_in[0].opt()],
                        outs=[a2a_out[0].opt()],
                    )

                # wo prefetch on gpsimd (idle but for collective triggers)
                for kt in range(nst):
                    nc.gpsimd.dma_start(wo_sb[:, nst * sc + kt, :], wo_r[nst * sc + kt])
                if sc == 0:
                    # second tiny collective: the first real AllToAll then
                    # runs on a fully warmed CC path
                    nc.gpsimd.collective_compute(
                        "AllToAll",
                        mybir.AluOpType.bypass,
                        replica_groups=[list(range(NC))],
                        ins=[warm_i.opt()],
                        outs=[warm_o.opt()],
                    )

            if causal:
                drain(attn_units(1, nsc - 2, a2a_in[1]))
                drain(attn_units(1, nsc - 1, a2a_in[1]))
            else:
                for qc in range(nsc):
                    drain(attn_units(0, qc, a2a_in[0]))
                nc.gpsimd.collective_compute(
                    "AllToAll",
                    mybir.AluOpType.bypass,
                    replica_groups=[list(range(NC))],
                    ins=[a2a_in[0].opt()],
                    outs=[a2a_out[0].opt()],
                )
                for qc in range(nsc):
                    drain(attn_units(1, qc, a2a_in[1]))
            nc.gpsimd.collective_compute(
                "AllToAll",
                mybir.AluOpType.bypass,
                replica_groups=[list(range(NC))],
                ins=[a2a_in[1].opt()],
                outs=[a2a_out[1].opt()],
            )

            # ---------------- output projection ----------------
            def wo_part(lhs_sb, col, final):
                for mt in range(rw // 128):
                    for nk in range(d // 512):
                        nsl = slice(nk * 512, (nk + 1) * 512)
                        w_ps = pss.tile([128, 512], F32, tag="pss", name=f"wps{col}_{mt}_{nk}")
                        for j in range(NC):
                            nc.tensor.matmul(
                                w_ps,
                                lhsT=lhs_sb[:, j, mt * 128 : (mt + 1) * 128],
                                rhs=wo_sb[:, 2 * j + col, nsl],
                                start=(j == 0),
                                stop=(j == NC - 1),
                            )
                        if not final:
                            nc.vector.tensor_copy(o_acc[:, mt, nsl], w_ps)
                        else:
                            o_sb = outp.tile([128, 512], F32, tag="osb", name=f"osb{mt}_{nk}")
                            nc.vector.tensor_add(o_sb, o_acc[:, mt, nsl], w_ps)
                            nc.sync.dma_start(
                                out[mt * 128 : (mt + 1) * 128, nsl], o_sb
                            )

            nc.sync.dma_start(lhs1_sb, a2a_out[0].rearrange("j p q -> p j q"))
            wo_part(lhs1_sb, 0, final=False)
            nc.sync.dma_start(lhs2_sb, a2a_out[1].rearrange("j p q -> p j q"))
            wo_part(lhs2_sb, 1, final=True)

    nc.compile()
    return nc


def host_prep(inputs, s=S, d=D, qc_w=512):
    f16 = np.float16
    x = np.ascontiguousarray(np.asarray(inputs["x"], dtype=np.float32)[0])
    wq = np.asarray(inputs["wq"], dtype=np.float32)
    wk = np.asarray(inputs["wk"], dtype=np.float32)
    wv = np.asarray(inputs["wv"], dtype=np.float32)
    wo = np.asarray(inputs["wo"], dtype=np.float32)
    ss = np.asarray(inputs["seq_scale"], dtype=np.float32).reshape(H)
    cos = np.asarray(inputs["freqs_cos"], dtype=np.float32)
    sin = np.asarray(inputs["freqs_sin"], dtype=np.float32)
    mask = np.asarray(inputs["mask"], dtype=np.float32)[0, 0]
    sll = np.asarray(inputs["section_log_len"], dtype=np.float32).reshape(s)

    nst = qc_w // 128
    zero = mask == 0.0
    causal = bool(
        np.array_equal(zero, np.tril(np.ones((s, s), bool)))
        and np.all(mask[~zero] <= -1e8)
    )
    shared = bool(np.all(ss == ss[0]))

    if causal:
        # staircase base [128, qc_w + nst*128]: diagonal pattern m (kpos
        # tile kt = nst*qc + m vs q chunk qc) is emb[:, qc_w-128m :
        # 2*qc_w-128m], i.e. emb[dk, c] = 1 iff (dk + 128m) <= dq with
        # dq = c - (qc_w - 128m)  <=>  dk <= c - qc_w
        w_em = qc_w + nst * 128
        ccol = np.arange(w_em)[None, :]
        crow = np.arange(128)[:, None]
        em_in = np.ascontiguousarray(
            (crow <= ccol - qc_w).astype(np.float32).astype(bf16)
        )
    else:
        em_in = np.ascontiguousarray(np.exp(np.minimum(mask, 0.0)).T.astype(bf16))

    perm = np.concatenate([np.arange(0, HD, 2), np.arange(1, HD, 2)])
    xt = np.ascontiguousarray(x.T.astype(f16))
    scale = sll / np.sqrt(HD)
    cksk = np.ascontiguousarray(np.concatenate([cos.T, sin.T], axis=0).astype(f16))
    wo_b = np.ascontiguousarray(wo.astype(bf16))

    in_maps = []
    for i in range(NC):
        wq_s = np.concatenate(
            [wq[:, CW * i + HD * h : CW * i + HD * (h + 1)][:, perm] for h in range(HL)],
            axis=1,
        )
        wk_s = np.concatenate(
            [wk[:, CW * i + HD * h : CW * i + HD * (h + 1)][:, perm] for h in range(HL)],
            axis=1,
        )
        wv_s = wv[:, CW * i : CW * (i + 1)]
        # per-head packed [cos; sin] scaled tables (one shared table when
        # seq_scale is uniform)
        cqt = np.concatenate(
            [cos.T * (scale * ss[HL * i])[None, :],
             sin.T * (scale * ss[HL * i])[None, :]], axis=0
        )
        m = {
            "xt": xt,
            "wqkv": np.ascontiguousarray(
                np.concatenate([wq_s, wk_s, wv_s], axis=1).astype(f16)
            ),
            "wo": wo_b,
            "cq": np.ascontiguousarray(cqt.astype(f16)),
            "cksk": cksk,
            "em": em_in,
        }
        if not shared:
            sqt = np.concatenate(
                [cos.T * (scale * ss[HL * i + 1])[None, :],
                 sin.T * (scale * ss[HL * i + 1])[None, :]], axis=0
            )
            m["sq"] = np.ascontiguousarray(sqt.astype(f16))
        in_maps.append(m)
    return in_maps, causal, shared


_NC_CACHE = {}


def _get_nc(causal, shared):
    key = (causal, shared)
    if key not in _NC_CACHE:
        _NC_CACHE[key] = build_nc(causal, shared)
    return _NC_CACHE[key]


def kernel(**inputs) -> np.ndarray:
    in_maps, causal, shared = host_prep(inputs)
    nc = _get_nc(causal, shared)
    res = run_bass_kernel_spmd(nc, in_maps, core_ids=list(range(NC)))
    rows = [res.results[i]["out"] for i in range(NC)]
    return np.concatenate(rows, axis=0)[None].astype(np.float32)
